# revision 1
# baseline (speedup 1.0000x reference)
"""Trainium2 Bass kernel for Conv2D_DT (distance-transform conv).

d(n,o,h,w) = || patch(n,:,h,w) - W[o,:] ||_2  with 3x3/pad1 im2col patches.

Strategy (8 NeuronCores, data-parallel over batch):
  - 4 images per core, processed as 2 pairs: image A on SBUF partitions
    0-63, image B on partitions 64-127 (channels = partition dim).
  - d2 = ||p||^2 + ||w||^2 - 2 p.w  accumulated fully in PSUM:
      * 9 shifted matmuls (taps) with lhsT = -2*W_tap, bf16 [K=64/image]
      * 1 matmul with lhsT = ones (f32r) over b = 3x3 box sum of x^2,
        which is the whole ||p||^2 term (channel sum via the contraction)
  - The two images' K=64 matmuls land on PE row-groups (0,0)/(64,0) and
    run concurrently -> full 128-row array utilization.
  - bf16 x-taps get FWL fast weight loads; the precision-critical box
    term streams f32r from fp32 squares; PSUM accumulates fp32.
  - epilogue: one ScalarE op  out = Sqrt(psum + w2[o])  then DMA out.
    (d2 >= ~200 for this data distribution, so Sqrt never sees <0.)
  - preprocessing (Square + 4 box adds) is emitted in row-halves and
    each chunk's b-matmul/epilogue is deferred 2 chunks so the PE queue
    front is x-taps only (no stall on b availability).
"""

import sys

_REPO = "/opt/trn_rl_repo"
if _REPO not in sys.path:
    sys.path.insert(0, _REPO)

import ml_dtypes
import numpy as np

import concourse.bass as bass  # noqa: F401
import concourse.mybir as mybir
import concourse.tile as tile
from concourse import bacc
from concourse.bass_utils import run_bass_kernel_spmd

# Problem geometry (hardcoded per harness contract).
N, C, H, W_DIM, O = 32, 64, 56, 56, 128
NCORES = 8
NL = N // NCORES  # images per core
NPAIR = NL // 2  # image pairs per core
HP = WP = 58  # zero-padded spatial dims
RCH = 8  # output rows per PSUM chunk
NCH = H // RCH  # 7 chunks per image
NXTAP = 9
DELAY = 3  # chunks between x-taps and b-slot/epilogue (8 PSUM banks)

F32 = mybir.dt.float32
F32R = mybir.dt.float32r
BF16 = mybir.dt.bfloat16

_PROGRAM = None


def _build_program():
    nc = bacc.Bacc(
        "TRN2",
        target_bir_lowering=False,
        debug=False,
        enable_asserts=False,
        num_devices=NCORES,
    )
    xs = nc.dram_tensor("xs", [NL, C, HP, WP], F32, kind="ExternalInput")
    xsb = nc.dram_tensor("xsb", [NL, C, HP, WP], BF16, kind="ExternalInput")
    lwb = nc.dram_tensor("lwb", [128, NXTAP, 128], BF16, kind="ExternalInput")
    lwo = nc.dram_tensor("lwo", [128, 128], F32R, kind="ExternalInput")
    w2 = nc.dram_tensor("w2", [128, 1], F32, kind="ExternalInput")
    out = nc.dram_tensor("out", [NL, O, H, W_DIM], F32, kind="ExternalOutput")

    with tile.TileContext(nc) as tc:
        with (
            tc.tile_pool(name="const", bufs=1) as cpool,
            tc.tile_pool(name="imgs", bufs=4) as ipool,
            tc.tile_pool(name="outs", bufs=4) as opool,
            tc.tile_pool(name="psum", bufs=8, space="PSUM") as ppool,
        ):
            lwbt = cpool.tile([128, NXTAP, 128], BF16)
            nc.sync.dma_start(out=lwbt[:], in_=lwb[:, :, :])
            lwot = cpool.tile([128, 128], F32R)
            nc.sync.dma_start(out=lwot[:], in_=lwo[:, :])
            w2t = cpool.tile([128, 1], F32)
            nc.sync.dma_start(out=w2t[:], in_=w2[:, :])

            # pair-halves: (padded row0, padded rows R); tt has R rows,
            # b has R-2 rows (output rows r0..r0+R-3)
            HALVES = ((0, 34, (0, 1, 2, 3)), (32, 26, (4, 5, 6)))

            def finish(item):
                ch, na, nb, psa, psb, bh, r0 = item
                h0 = ch * RCH
                lb = h0 - r0
                for half, ps in ((slice(0, 64), psa), (slice(64, 128), psb)):
                    nc.tensor.matmul(
                        ps[:],
                        lwot[half, :],
                        bh[half, lb : lb + RCH, :],
                        start=False,
                        stop=True,
                    )
                for ps, n_img in ((psa, na), (psb, nb)):
                    ot = opool.tile([128, RCH, W_DIM], F32, tag="ot")
                    nc.scalar.activation(
                        out=ot[:],
                        in_=ps[:],
                        func=mybir.ActivationFunctionType.Sqrt,
                        bias=w2t[:],
                        scale=1.0,
                    )
                    nc.sync.dma_start(
                        out=out[n_img, :, h0 : h0 + RCH, :], in_=ot[:]
                    )

            pending = []
            for p in range(NPAIR):
                na, nb = 2 * p, 2 * p + 1
                halves = []
                for r0, R, chs in HALVES:
                    xbh = ipool.tile([128, R, WP], BF16, tag="xbh")
                    nc.sync.dma_start(
                        out=xbh[0:64, :, :], in_=xsb[na, :, r0 : r0 + R, :]
                    )
                    nc.sync.dma_start(
                        out=xbh[64:128, :, :], in_=xsb[nb, :, r0 : r0 + R, :]
                    )
                    xph = ipool.tile([128, R, WP], F32, tag="xph")
                    nc.sync.dma_start(
                        out=xph[0:64, :, :], in_=xs[na, :, r0 : r0 + R, :]
                    )
                    nc.sync.dma_start(
                        out=xph[64:128, :, :], in_=xs[nb, :, r0 : r0 + R, :]
                    )
                    sqh = ipool.tile([128, R, WP], F32, tag="sqh")
                    nc.scalar.activation(
                        out=sqh[:],
                        in_=xph[:],
                        func=mybir.ActivationFunctionType.Square,
                    )
                    uh = ipool.tile([128, R, W_DIM], F32, tag="uh")
                    nc.vector.tensor_add(uh[:], sqh[:, :, 0:56], sqh[:, :, 1:57])
                    tth = ipool.tile([128, R, W_DIM], F32, tag="tth")
                    nc.vector.tensor_add(tth[:], uh[:], sqh[:, :, 2:58])
                    vh = ipool.tile([128, R - 2, W_DIM], F32, tag="vh")
                    nc.vector.tensor_add(
                        vh[:], tth[:, 0 : R - 2, :], tth[:, 1 : R - 1, :]
                    )
                    bh = ipool.tile([128, R - 2, W_DIM], F32R, tag="bh")
                    nc.vector.tensor_add(bh[:], vh[:], tth[:, 2:R, :])
                    halves.append((r0, chs, xbh, bh))

                for r0, chs, xbh, bh in halves:
                    for ch in chs:
                        lh = ch * RCH - r0  # chunk's first row, local to half
                        psa = ppool.tile([128, RCH, W_DIM], F32, tag="ps")
                        psb = ppool.tile([128, RCH, W_DIM], F32, tag="ps")
                        for slot in range(NXTAP):
                            kh, kw = divmod(slot, 3)
                            rhs = xbh[:, lh + kh : lh + kh + RCH, kw : kw + 56]
                            st = slot == 0
                            nc.tensor.matmul(
                                psa[:],
                                lwbt[0:64, slot, :],
                                rhs[0:64],
                                start=st,
                                stop=False,
                            )
                            nc.tensor.matmul(
                                psb[:],
                                lwbt[64:128, slot, :],
                                rhs[64:128],
                                start=st,
                                stop=False,
                            )
                        pending.append((ch, na, nb, psa, psb, bh, r0))
                        if len(pending) > DELAY:
                            finish(pending.pop(0))
            for item in pending:
                finish(item)
    nc.compile()
    return nc


def _host_weights(W):
    """bf16 x-tap lhsT [128, 9, 128] (dup on both halves), f32r ones, w2."""
    W = np.asarray(W, np.float32)
    lhs = np.zeros((128, NXTAP, 128), np.float32)
    cidx = np.arange(C)
    for kh in range(3):
        for kw in range(3):
            slot = kh * 3 + kw
            blk = (-2.0 * W[:, cidx * 9 + kh * 3 + kw]).T  # [C, O]
            lhs[0:64, slot, :] = blk
            lhs[64:128, slot, :] = blk
    lwo = np.ones((128, 128), np.float32)
    w2 = (W * W).sum(axis=1).astype(np.float32).reshape(128, 1)
    return lhs.astype(ml_dtypes.bfloat16), lwo, w2


def get_program():
    global _PROGRAM
    if _PROGRAM is None:
        _PROGRAM = _build_program()
    return _PROGRAM


def make_in_maps(x, W):
    x = np.asarray(x, np.float32)
    xpad = np.zeros((N, C, HP, WP), np.float32)
    xpad[:, :, 1 : H + 1, 1 : W_DIM + 1] = x
    xpadb = xpad.astype(ml_dtypes.bfloat16)
    lwb, lwo, w2 = _host_weights(W)
    return [
        {
            "xs": xpad[i * NL : (i + 1) * NL],
            "xsb": xpadb[i * NL : (i + 1) * NL],
            "lwb": lwb,
            "lwo": lwo,
            "w2": w2,
        }
        for i in range(NCORES)
    ]


def kernel(x, W):
    nc = get_program()
    in_maps = make_in_maps(x, W)
    res = run_bass_kernel_spmd(nc, in_maps, list(range(NCORES)))
    outs = [res.results[i]["out"] for i in range(NCORES)]
    return np.concatenate(outs, axis=0)



# revision 2
# speedup vs baseline: 1.1791x; 1.1791x over previous
"""Trainium2 Bass kernel for Conv2D_DT (distance-transform conv).

d(n,o,h,w) = || patch(n,:,h,w) - W[o,:] ||_2  with 3x3/pad1 im2col patches.

v2 strategy (8 NeuronCores, data-parallel over batch, 4 images/core):
  - d2 = ||p||^2 - 2 p.w + ||w||^2 accumulated in PSUM per 8-row chunk:
      * 6 fp8 DoubleRow matmuls per image-chunk: taps (0,kw)+(1,kw) are
        row-pairs (pair stride = one padded row, verified on HW); taps
        (2,kw) ride a DR slot with a zero partner weight.  fp8 runs the
        PE at 2x (0.5 cyc/row).
      * 1 bf16 ones-matmul adds the precomputed per-channel 3x3 box sum
        of x^2 (the whole ||p||^2 term via the K=64 contraction).
  - x ships as ONE fp8 copy [4,64,59,60]; squares are computed on-chip:
    GpSimd squares fp8->bf16 (otherwise idle engine), DVE does the 4
    box-sum adds in bf16 (2x mode).
  - Two images run concurrently on PE row groups (0,0)/(64,0).
  - Epilogue: ScalarE Sqrt(psum + w2) -> fp16; host upcasts to fp32
    (fp16 dist rel err ~5e-4, far under the 2e-2 gate).  Output DMA is
    halved vs fp32.
  - Weight-stationary rounds: slot-outer over 2 chunks x 2 images per
    PSUM generation amortizes DoubleRow LDWEIGHTS (no FWL in DR mode).
"""

import sys

_REPO = "/opt/trn_rl_repo"
if _REPO not in sys.path:
    sys.path.insert(0, _REPO)

import ml_dtypes
import numpy as np

import concourse.bass as bass  # noqa: F401
import concourse.mybir as mybir
import concourse.tile as tile
from concourse import bacc
from concourse.bass_utils import run_bass_kernel_spmd

# Problem geometry (hardcoded per harness contract).
N, C, H, W_DIM, O = 32, 64, 56, 56, 128
NCORES = 8
NL = N // NCORES  # images per core
NPAIR = NL // 2
HP, WP = 59, 60  # padded x dims (row 58 / cols 57..59 are junk-read pads)
RCH = 8  # output rows per PSUM chunk
NCH = H // RCH  # 7 chunks per image
NSLOT = 6  # fp8 DoubleRow tap slots

F32 = mybir.dt.float32
F16 = mybir.dt.float16
BF16 = mybir.dt.bfloat16
F8 = mybir.dt.float8e4
DRM = mybir.MatmulPerfMode.DoubleRow

# halves: (x row base, x rows, tth row base, tth rows, chunks)
HALVES = (
    (0, 27, 0, 26, (0, 1, 2)),
    (24, 35, 24, 34, (3, 4, 5, 6)),
)
# per half: (sq strip row ranges [local], bh strip row ranges [local])
STRIPS = {
    0: (((0, 10), (10, 26)), ((0, 8), (8, 24))),
    1: (((0, 10), (10, 34)), ((0, 8), (8, 32))),
}
# x DMA row segments [local]
XSEGS = {0: ((0, 11), (11, 27)), 1: ((0, 11), (11, 35))}

_PROGRAM = None


def _dr_rhs(xh, hp, row0, col0):
    """[64, 2, RCH, 56] fp8 AP with pair stride = one padded row (WP)."""
    ap = xh[hp : hp + 64, row0 : row0 + RCH, col0 : col0 + 56].unsqueeze(1)
    raw = [list(p) for p in ap.ap]
    raw[1] = [WP, 2]
    ap.ap = mybir.VecI64Pair(raw)
    return ap


def _build_program():
    nc = bacc.Bacc(
        "TRN2",
        target_bir_lowering=False,
        debug=False,
        enable_asserts=False,
        num_devices=NCORES,
    )
    x8 = nc.dram_tensor("x8", [NL, C, HP, WP], F8, kind="ExternalInput")
    lw8 = nc.dram_tensor("lw8", [128, NSLOT, 2, 128], F8, kind="ExternalInput")
    ones = nc.dram_tensor("ones", [128, 128], BF16, kind="ExternalInput")
    w2 = nc.dram_tensor("w2", [128, 1], F32, kind="ExternalInput")
    out = nc.dram_tensor("out", [NL, O, H, W_DIM], F16, kind="ExternalOutput")

    with tile.TileContext(nc) as tc:
        with (
            tc.tile_pool(name="const", bufs=1) as cpool,
            tc.tile_pool(name="imgs", bufs=2) as ipool,
            tc.tile_pool(name="outs", bufs=4) as opool,
            tc.tile_pool(name="psum", bufs=2, space="PSUM") as ppool,
        ):
            lw8t = cpool.tile([128, NSLOT, 2, 128], F8)
            nc.sync.dma_start(out=lw8t[:], in_=lw8[:, :, :, :])
            onest = cpool.tile([128, 128], BF16)
            nc.sync.dma_start(out=onest[:], in_=ones[:, :])
            w2t = cpool.tile([128, 1], F32)
            nc.sync.dma_start(out=w2t[:], in_=w2[:, :])

            def preprocess(p):
                """DMA + squares + box sums for pair p. Returns per-half
                (x tile, bh tile, x row base, tth row base)."""
                na, nb = 2 * p, 2 * p + 1
                halves = []
                for hi, (r0x, RX, t0, RT, _chunks) in enumerate(HALVES):
                    xh = ipool.tile([128, RX, WP], F8, tag=f"xh{hi}")
                    for s0, s1 in XSEGS[hi]:
                        nc.sync.dma_start(
                            out=xh[0:64, s0:s1, :],
                            in_=x8[na, :, r0x + s0 : r0x + s1, :],
                        )
                        nc.sync.dma_start(
                            out=xh[64:128, s0:s1, :],
                            in_=x8[nb, :, r0x + s0 : r0x + s1, :],
                        )
                    sqh = ipool.tile([128, RT, 58], BF16, tag=f"sq{hi}")
                    tth = ipool.tile([128, RT, 56], BF16, tag=f"tt{hi}")
                    bhh = ipool.tile([128, RT - 2, 56], BF16, tag=f"bh{hi}")
                    sq_strips, bh_strips = STRIPS[hi]
                    for (q0, q1), (b0, b1) in zip(sq_strips, bh_strips):
                        nc.gpsimd.tensor_mul(
                            sqh[:, q0:q1, :], xh[:, q0:q1, 0:58], xh[:, q0:q1, 0:58]
                        )
                        uh = ipool.tile([128, q1 - q0, 56], BF16, tag=f"u{hi}{q0}")
                        nc.vector.tensor_add(
                            uh[:], sqh[:, q0:q1, 0:56], sqh[:, q0:q1, 1:57]
                        )
                        nc.vector.tensor_add(
                            tth[:, q0:q1, :], uh[:], sqh[:, q0:q1, 2:58]
                        )
                        vh = ipool.tile([128, b1 - b0, 56], BF16, tag=f"v{hi}{b0}")
                        nc.vector.tensor_add(
                            vh[:], tth[:, b0:b1, :], tth[:, b0 + 1 : b1 + 1, :]
                        )
                        nc.vector.tensor_add(
                            bhh[:, b0:b1, :], vh[:], tth[:, b0 + 2 : b1 + 2, :]
                        )
                    halves.append((xh, bhh, r0x, t0))
                return na, nb, halves

            def rounds(p, halves, chunk_groups):
                """Matmul rounds + epilogue for pair p."""
                na, nb = 2 * p, 2 * p + 1

                def half_of(ch):
                    return 0 if ch in HALVES[0][4] else 1

                for group in chunk_groups:
                    nch = len(group)
                    psA = ppool.tile([128, 2, 512], F32, tag="psA")
                    psB = ppool.tile([128, 2, 512], F32, tag="psB")
                    for s in range(NSLOT):
                        kh0 = 0 if s < 3 else 2
                        kw = s % 3
                        st = s == 0
                        for hp, ps in ((0, psA), (64, psB)):
                            for ci, ch in enumerate(group):
                                xh, _bh, r0x, _t0 = halves[half_of(ch)]
                                lh = ch * RCH - r0x
                                nc.tensor.matmul(
                                    ps[:, ci, 0:448],
                                    lw8t[hp : hp + 64, s],
                                    _dr_rhs(xh, hp, lh + kh0, kw),
                                    start=st,
                                    stop=False,
                                    perf_mode=DRM,
                                    tile_position=(hp, 0),
                                )
                    for hp, ps in ((0, psA), (64, psB)):
                        for ci, ch in enumerate(group):
                            _xh, bhh, _r0x, t0 = halves[half_of(ch)]
                            lb = ch * RCH - t0
                            nc.tensor.matmul(
                                ps[:, ci, 0:448],
                                onest[hp : hp + 64, :],
                                bhh[hp : hp + 64, lb : lb + RCH, :],
                                start=False,
                                stop=True,
                                tile_position=(hp, 0),
                            )
                    h0 = group[0] * RCH
                    for n_img, ps in ((na, psA), (nb, psB)):
                        ot = opool.tile([128, nch, 448], F16, tag="ot")
                        nc.scalar.activation(
                            out=ot[:],
                            in_=ps[:, 0:nch, 0:448],
                            func=mybir.ActivationFunctionType.Sqrt,
                            bias=w2t[:],
                            scale=1.0,
                        )
                        nc.sync.dma_start(
                            out=out[n_img, :, h0 : h0 + RCH * nch, :], in_=ot[:]
                        )

            na0, nb0, halves0 = preprocess(0)
            rounds(0, halves0, [(0,)])
            na1, nb1, halves1 = preprocess(1)
            rounds(0, halves0, [(1, 2), (3, 4), (5, 6)])
            rounds(1, halves1, [(0,), (1, 2), (3, 4), (5, 6)])
    nc.compile()
    return nc


def _host_weights(W):
    """fp8 DR tap weights [128,6,2,128], bf16 ones, f32 w2."""
    W = np.asarray(W, np.float32)
    cidx = np.arange(C)
    lw = np.zeros((128, NSLOT, 2, 128), np.float32)
    for s in range(NSLOT):
        kh0 = 0 if s < 3 else 2
        kw = s % 3
        blk0 = (-2.0 * W[:, cidx * 9 + kh0 * 3 + kw]).T  # [C, O]
        lw[0:64, s, 0, :] = blk0
        lw[64:128, s, 0, :] = blk0
        if s < 3:
            blk1 = (-2.0 * W[:, cidx * 9 + 1 * 3 + kw]).T
            lw[0:64, s, 1, :] = blk1
            lw[64:128, s, 1, :] = blk1
    ones = np.ones((128, 128), np.float32)
    w2 = (W * W).sum(axis=1).astype(np.float32).reshape(128, 1)
    return (
        lw.astype(ml_dtypes.float8_e4m3),
        ones.astype(ml_dtypes.bfloat16),
        w2,
    )


def get_program():
    global _PROGRAM
    if _PROGRAM is None:
        _PROGRAM = _build_program()
    return _PROGRAM


def make_in_maps(x, W):
    x = np.asarray(x, np.float32)
    xpad = np.zeros((N, C, HP, WP), np.float32)
    xpad[:, :, 1 : H + 1, 1 : W_DIM + 1] = x
    x8 = xpad.astype(ml_dtypes.float8_e4m3)
    lw8, ones, w2 = _host_weights(W)
    return [
        {
            "x8": x8[i * NL : (i + 1) * NL],
            "lw8": lw8,
            "ones": ones,
            "w2": w2,
        }
        for i in range(NCORES)
    ]


def kernel(x, W):
    nc = get_program()
    in_maps = make_in_maps(x, W)
    res = run_bass_kernel_spmd(nc, in_maps, list(range(NCORES)))
    outs = [np.asarray(res.results[i]["out"]).astype(np.float32) for i in range(NCORES)]
    return np.concatenate(outs, axis=0)


# revision 6
# speedup vs baseline: 1.4364x; 1.2182x over previous
"""Trainium2 Bass kernel for Conv2D_DT (distance-transform conv).

d(n,o,h,w) = || patch(n,:,h,w) - W[o,:] ||_2  with 3x3/pad1 im2col patches.

v2 strategy (8 NeuronCores, data-parallel over batch, 4 images/core):
  - d2 = ||p||^2 - 2 p.w + ||w||^2 accumulated in PSUM per 8-row chunk:
      * 6 fp8 DoubleRow matmuls per image-chunk: taps (0,kw)+(1,kw) are
        row-pairs (pair stride = one padded row, verified on HW); taps
        (2,kw) ride a DR slot with a zero partner weight.  fp8 runs the
        PE at 2x (0.5 cyc/row).
      * 1 bf16 ones-matmul adds the precomputed per-channel 3x3 box sum
        of x^2 (the whole ||p||^2 term via the K=64 contraction).
  - x ships as ONE fp8 copy [4,64,59,60]; squares are computed on-chip:
    GpSimd squares fp8->bf16 (otherwise idle engine), DVE does the 4
    box-sum adds in bf16 (2x mode).
  - Two images run concurrently on PE row groups (0,0)/(64,0).
  - Epilogue: ScalarE Sqrt(psum + w2) -> fp16; host upcasts to fp32
    (fp16 dist rel err ~5e-4, far under the 2e-2 gate).  Output DMA is
    halved vs fp32.
  - Weight-stationary rounds: slot-outer over 2 chunks x 2 images per
    PSUM generation amortizes DoubleRow LDWEIGHTS (no FWL in DR mode).
"""

import sys

_REPO = "/opt/trn_rl_repo"
if _REPO not in sys.path:
    sys.path.insert(0, _REPO)

import ml_dtypes
import numpy as np

import concourse.bass as bass  # noqa: F401
import concourse.mybir as mybir
import concourse.tile as tile
from concourse import bacc
from concourse.bass_utils import run_bass_kernel_spmd

# Problem geometry (hardcoded per harness contract).
N, C, H, W_DIM, O = 32, 64, 56, 56, 128
NCORES = 8
NL = N // NCORES  # images per core
NPAIR = NL // 2
HP, WP = 59, 60  # padded x dims (row 58 / cols 57..59 are junk-read pads)
RCH = 8  # output rows per PSUM chunk
NCH = H // RCH  # 7 chunks per image
NSLOT = 6  # fp8 DoubleRow tap slots

F32 = mybir.dt.float32
F16 = mybir.dt.float16
BF16 = mybir.dt.bfloat16
F8 = mybir.dt.float8e4
DRM = mybir.MatmulPerfMode.DoubleRow

# halves: (x row base, x rows, tth row base, tth rows, chunks)
HALVES = (
    (0, 27, 0, 26, (0, 1, 2)),
    (24, 35, 24, 34, (3, 4, 5, 6)),
)
# per half: (sq strip row ranges [local], bh strip row ranges [local])
STRIPS = {
    0: (((0, 10), (10, 26)), ((0, 8), (8, 24))),
    1: (((0, 10), (10, 34)), ((0, 8), (8, 32))),
}
# x DMA row segments [local]
XSEGS = {0: ((0, 11), (11, 27)), 1: ((0, 11), (11, 35))}

_PROGRAM = None


def _dr_rhs(xh, hp, row0, col0):
    """[64, 2, RCH, 56] fp8 AP with pair stride = one padded row (WP)."""
    ap = xh[hp : hp + 64, row0 : row0 + RCH, col0 : col0 + 56].unsqueeze(1)
    raw = [list(p) for p in ap.ap]
    raw[1] = [WP, 2]
    ap.ap = mybir.VecI64Pair(raw)
    return ap


def _build_program():
    nc = bacc.Bacc(
        "TRN2",
        target_bir_lowering=False,
        debug=False,
        enable_asserts=False,
        num_devices=NCORES,
    )
    x8 = nc.dram_tensor("x8", [NL, C, HP, WP], F8, kind="ExternalInput")
    lw8 = nc.dram_tensor("lw8", [128, NSLOT, 2, 128], F8, kind="ExternalInput")
    ones = nc.dram_tensor("ones", [128, 128], BF16, kind="ExternalInput")
    w2 = nc.dram_tensor("w2", [128, 1], F32, kind="ExternalInput")
    out = nc.dram_tensor("out", [NL, O, H, W_DIM], F16, kind="ExternalOutput")

    with tile.TileContext(nc) as tc:
        with (
            tc.tile_pool(name="const", bufs=1) as cpool,
            tc.tile_pool(name="imgs", bufs=2) as ipool,
            tc.tile_pool(name="outs", bufs=4) as opool,
            tc.tile_pool(name="psum", bufs=2, space="PSUM") as ppool,
        ):
            lw8t = cpool.tile([128, NSLOT, 2, 128], F8)
            onest = cpool.tile([128, 128], BF16)
            w2t = cpool.tile([128, 1], F32)

            def load_consts():
                nc.sync.dma_start(out=lw8t[:], in_=lw8[:, :, :, :])
                nc.sync.dma_start(out=onest[:], in_=ones[:, :])
                nc.sync.dma_start(out=w2t[:], in_=w2[:, :])

            def preprocess(p):
                """DMA + squares + box sums for pair p. Returns per-half
                (x tile, bh tile, x row base, tth row base)."""
                na, nb = 2 * p, 2 * p + 1
                halves = []
                for hi, (r0x, RX, t0, RT, _chunks) in enumerate(HALVES):
                    xh = ipool.tile([128, RX, WP], F8, tag=f"xh{hi}")
                    for s0, s1 in XSEGS[hi]:
                        nc.sync.dma_start(
                            out=xh[0:64, s0:s1, :],
                            in_=x8[na, :, r0x + s0 : r0x + s1, :],
                        )
                        nc.sync.dma_start(
                            out=xh[64:128, s0:s1, :],
                            in_=x8[nb, :, r0x + s0 : r0x + s1, :],
                        )
                    sqh = ipool.tile([128, RT, 58], BF16, tag=f"sq{hi}")
                    tth = ipool.tile([128, RT, 56], BF16, tag=f"tt{hi}")
                    bhh = ipool.tile([128, RT - 2, 56], BF16, tag=f"bh{hi}")
                    sq_strips, bh_strips = STRIPS[hi]
                    for si, ((q0, q1), (b0, b1)) in enumerate(
                        zip(sq_strips, bh_strips)
                    ):
                        if si == 0:
                            # first (chunk-blocking) strip squares on the
                            # otherwise-idle ScalarE for lower latency
                            nc.scalar.activation(
                                out=sqh[:, q0:q1, :],
                                in_=xh[:, q0:q1, 0:58],
                                func=mybir.ActivationFunctionType.Square,
                            )
                        else:
                            nc.gpsimd.tensor_mul(
                                sqh[:, q0:q1, :],
                                xh[:, q0:q1, 0:58],
                                xh[:, q0:q1, 0:58],
                            )
                        uh = ipool.tile([128, q1 - q0, 56], BF16, tag=f"u{hi}{q0}")
                        nc.vector.tensor_add(
                            uh[:], sqh[:, q0:q1, 0:56], sqh[:, q0:q1, 1:57]
                        )
                        nc.vector.tensor_add(
                            tth[:, q0:q1, :], uh[:], sqh[:, q0:q1, 2:58]
                        )
                        vh = ipool.tile([128, b1 - b0, 56], BF16, tag=f"v{hi}{b0}")
                        nc.vector.tensor_add(
                            vh[:], tth[:, b0:b1, :], tth[:, b0 + 1 : b1 + 1, :]
                        )
                        nc.vector.tensor_add(
                            bhh[:, b0:b1, :], vh[:], tth[:, b0 + 2 : b1 + 2, :]
                        )
                    halves.append((xh, bhh, r0x, t0))
                return na, nb, halves

            def rounds(p, halves, chunk_groups):
                """Matmul rounds + epilogue for pair p."""
                na, nb = 2 * p, 2 * p + 1

                def half_of(ch):
                    return 0 if ch in HALVES[0][4] else 1

                for group in chunk_groups:
                    nch = len(group)
                    psA = ppool.tile([128, 2, 512], F32, tag="psA")
                    psB = ppool.tile([128, 2, 512], F32, tag="psB")
                    for s in range(NSLOT):
                        kh0 = 0 if s < 3 else 2
                        kw = s % 3
                        st = s == 0
                        # alternate row groups so each LDWEIGHTS overlaps
                        # the other group's in-flight matmul
                        for ci, ch in enumerate(group):
                            xh, _bh, r0x, _t0 = halves[half_of(ch)]
                            lh = ch * RCH - r0x
                            for hp, ps in ((0, psA), (64, psB)):
                                nc.tensor.matmul(
                                    ps[:, ci, 0:448],
                                    lw8t[hp : hp + 64, s],
                                    _dr_rhs(xh, hp, lh + kh0, kw),
                                    start=st,
                                    stop=False,
                                    perf_mode=DRM,
                                    tile_position=(hp, 0),
                                )
                    for ci, ch in enumerate(group):
                        _xh, bhh, _r0x, t0 = halves[half_of(ch)]
                        lb = ch * RCH - t0
                        for hp, ps in ((0, psA), (64, psB)):
                            nc.tensor.matmul(
                                ps[:, ci, 0:448],
                                onest[hp : hp + 64, :],
                                bhh[hp : hp + 64, lb : lb + RCH, :],
                                start=False,
                                stop=True,
                                tile_position=(hp, 0),
                            )
                    h0 = group[0] * RCH
                    for n_img, ps in ((na, psA), (nb, psB)):
                        ot = opool.tile([128, nch, 448], F16, tag="ot")
                        nc.scalar.activation(
                            out=ot[:],
                            in_=ps[:, 0:nch, 0:448],
                            func=mybir.ActivationFunctionType.Sqrt,
                            bias=w2t[:],
                            scale=1.0,
                        )
                        nc.sync.dma_start(
                            out=out[n_img, :, h0 : h0 + RCH * nch, :], in_=ot[:]
                        )

            na0, nb0, halves0 = preprocess(0)
            load_consts()
            rounds(0, halves0, [(0,)])
            na1, nb1, halves1 = preprocess(1)
            rounds(0, halves0, [(1, 2), (3, 4), (5, 6)])
            rounds(1, halves1, [(0,), (1, 2), (3, 4), (5, 6)])
    nc.compile()
    return nc


def _host_weights(W):
    """fp8 DR tap weights [128,6,2,128], bf16 ones, f32 w2."""
    W = np.asarray(W, np.float32)
    cidx = np.arange(C)
    lw = np.zeros((128, NSLOT, 2, 128), np.float32)
    for s in range(NSLOT):
        kh0 = 0 if s < 3 else 2
        kw = s % 3
        blk0 = (-2.0 * W[:, cidx * 9 + kh0 * 3 + kw]).T  # [C, O]
        lw[0:64, s, 0, :] = blk0
        lw[64:128, s, 0, :] = blk0
        if s < 3:
            blk1 = (-2.0 * W[:, cidx * 9 + 1 * 3 + kw]).T
            lw[0:64, s, 1, :] = blk1
            lw[64:128, s, 1, :] = blk1
    ones = np.ones((128, 128), np.float32)
    w2 = (W * W).sum(axis=1).astype(np.float32).reshape(128, 1)
    return (
        lw.astype(ml_dtypes.float8_e4m3),
        ones.astype(ml_dtypes.bfloat16),
        w2,
    )


def get_program():
    global _PROGRAM
    if _PROGRAM is None:
        _PROGRAM = _build_program()
    return _PROGRAM


def make_in_maps(x, W):
    x = np.asarray(x, np.float32)
    xpad = np.zeros((N, C, HP, WP), np.float32)
    xpad[:, :, 1 : H + 1, 1 : W_DIM + 1] = x
    x8 = xpad.astype(ml_dtypes.float8_e4m3)
    lw8, ones, w2 = _host_weights(W)
    return [
        {
            "x8": x8[i * NL : (i + 1) * NL],
            "lw8": lw8,
            "ones": ones,
            "w2": w2,
        }
        for i in range(NCORES)
    ]


def kernel(x, W):
    nc = get_program()
    in_maps = make_in_maps(x, W)
    res = run_bass_kernel_spmd(nc, in_maps, list(range(NCORES)))
    outs = [np.asarray(res.results[i]["out"]).astype(np.float32) for i in range(NCORES)]
    return np.concatenate(outs, axis=0)
